# revision 11
# baseline (speedup 1.0000x reference)
"""FP8-per-channel fake-quantized linear, 8-core Trainium2 (Bass/Tile).

Math (reference, all fp32):
    s      = max(max|x| / 448, 1e-12)                 # global input scale
    x_q    = round(clip(x / s, +-448))                # integers in [-448, 448]
    ws[o]  = max(max_k|w[o,k]| / 448, 1e-12)          # per-out-channel scale
    w_q    = round(clip(w / ws[:,None], +-448))       # integers in [-448, 448]
    out    = (x_q @ w_q.T) * (s * ws) + bias

Key facts exploited here:
  * x_q / w_q are integers with |v| <= 448 -> exactly representable in fp16.
    One fp16 matmul with fp32 PSUM accumulation reproduces the integer GEMM
    exactly (products <= 448^2 and partial sums << 2^24).
  * round-half-to-even == fp32 "+ 1.5*2^23 then - 1.5*2^23" trick.
  * Sharding: tokens (16384 -> 2048/core), weight replicated; the global
    input amax needs one tiny AllGather (+local max) across the 8 cores.
  * Both matmul operands need K on partitions; the transposes are done with
    the DMA XBAR on the quantized fp16 tiles (fp32 has no DMA-transpose).
"""

import numpy as np
from contextlib import ExitStack

import concourse.bass as bass
import concourse.tile as tile
from concourse import bacc, mybir
from concourse import bass_isa
from concourse.bass import ts
from concourse.bass_utils import run_bass_kernel_spmd
from concourse.masks import make_identity

F32 = mybir.dt.float32
F16 = mybir.dt.float16
ALU = mybir.AluOpType
ACTF = mybir.ActivationFunctionType
AX = mybir.AxisListType

FP8_MAX = 448.0
# reference clamps the scale at 1e-12; clamping amax at 448e-12 is identical
AMAX_FLOOR = 448e-12
RND_C = 12582912.0  # 1.5 * 2^23: fp32 round-to-nearest-even magic constant
P = 128


def _quantize(nc, pools, src_f32, dst_f16, inv_scale_ap):
    """dst_f16 = round_half_even(src_f32 * inv_scale) as fp16.

    inv_scale_ap: [P, 1] per-partition fp32 reciprocal of the quant scale.
    Split across ACT (affine) + DVE (subtract & downcast) to balance engines.
    """
    p, f = src_f32.shape
    tmp = pools["qtmp"].tile([p, f], F32, tag="qtmp")
    # ACT: tmp = src * inv_scale + C   (rounds to integer at the add)
    nc.scalar.activation(tmp[:], src_f32[:], ACTF.Copy, bias=RND_C, scale=inv_scale_ap)
    # DVE: dst = fp16(tmp - C)         (exact: integer of magnitude <= 448)
    nc.vector.tensor_scalar(dst_f16[:], tmp[:], RND_C, None, ALU.subtract)


def build_nc(n_cores=8, t_local=2048, k_dim=2048, o_dim=2048):
    """Build the per-core Bass program (SPMD: same program on every core)."""
    nc = bacc.Bacc(
        "TRN2", target_bir_lowering=False, debug=False, num_devices=n_cores
    )
    x_d = nc.dram_tensor("x", [t_local, k_dim], F32, kind="ExternalInput")
    w_d = nc.dram_tensor("w", [o_dim, k_dim], F32, kind="ExternalInput")
    b_d = nc.dram_tensor("b", [o_dim], F32, kind="ExternalInput")
    out_d = nc.dram_tensor("out", [t_local, o_dim], F32, kind="ExternalOutput")

    # collective bounce buffers (DRAM; output Shared for the fast path)
    cc_in = nc.dram_tensor("cc_in", [1, 1], F32)
    cc_out = nc.dram_tensor(
        "cc_out", [1, n_cores], F32,
        addr_space="Shared" if n_cores > 4 else "Local",
    )

    with tile.TileContext(nc) as tc:
        _body(tc, x_d.ap(), w_d.ap(), b_d.ap(), out_d.ap(), cc_in.ap(), cc_out.ap(),
              n_cores=n_cores)
    nc.compile()
    return nc


def _body(tc, x, w, b, out, cc_in, cc_out, n_cores):
    nc = tc.nc
    t_local, k_dim = x.shape
    o_dim = w.shape[0]
    TT = t_local // P      # token tiles
    KO = k_dim // P        # contraction subtiles
    OJ = o_dim // P        # weight row tiles
    N_TILE = 512           # psum free width
    OO = o_dim // N_TILE   # output column tiles

    with ExitStack() as ctx:
        singles = ctx.enter_context(tc.tile_pool(name="singles", bufs=1))
        xin = ctx.enter_context(tc.tile_pool(name="xin", bufs=3))
        win = ctx.enter_context(tc.tile_pool(name="win", bufs=2))
        qtmp = ctx.enter_context(tc.tile_pool(name="qtmp", bufs=2))
        q16 = ctx.enter_context(tc.tile_pool(name="q16", bufs=3))
        xqt = ctx.enter_context(tc.tile_pool(name="xqt", bufs=3))
        outp = ctx.enter_context(tc.tile_pool(name="outp", bufs=4))
        small = ctx.enter_context(tc.tile_pool(name="small", bufs=4))
        psum = ctx.enter_context(tc.tile_pool(name="psum", bufs=7, space="PSUM"))
        psum1 = ctx.enter_context(tc.tile_pool(name="psum1", bufs=1, space="PSUM"))
        pools = {"qtmp": qtmp}

        # ---------------- Phase A: local |x| max -> global via AllGather ----
        xam = singles.tile([P, TT], F32)
        for tt in range(TT):
            xt = xin.tile([P, k_dim], F32, tag="xt")
            nc.sync.dma_start(xt[:], x[ts(tt, P), :])
            nc.vector.tensor_reduce(
                xam[:, tt : tt + 1], xt[:], axis=AX.X, op=ALU.max,
                apply_absolute_value=True,
            )
        ident = singles.tile([P, P], F32)
        make_identity(nc, ident[:])

        xam1 = singles.tile([P, 1], F32)
        nc.vector.tensor_reduce(xam1[:], xam[:], axis=AX.X, op=ALU.max)
        # 128 partitions -> 1: PE transpose [128,1] -> [1,128], then reduce
        xamT_ps = psum1.tile([1, P], F32, name="xamT_ps", tag="psmisc")
        nc.tensor.transpose(xamT_ps[:], xam1[:], ident[:])
        xamT = singles.tile([1, P], F32)
        nc.vector.tensor_copy(xamT[:], xamT_ps[:])
        loc1 = singles.tile([1, 1], F32)
        nc.vector.tensor_reduce(loc1[:], xamT[:], axis=AX.X, op=ALU.max)
        nc.sync.dma_start(cc_in, loc1[:])
        nc.gpsimd.collective_compute(
            "AllGather",
            ALU.bypass,
            replica_groups=[list(range(n_cores))],
            ins=[cc_in.opt()],
            outs=[cc_out.opt()],
        )
        # broadcast-load the 8 per-core amaxes to every partition, reduce
        am8 = singles.tile([P, n_cores], F32)
        nc.sync.dma_start(am8[:], cc_out.to_broadcast((P, n_cores)))
        gmaxP = singles.tile([P, 1], F32)
        nc.vector.tensor_reduce(gmaxP[:], am8[:], axis=AX.X, op=ALU.max)

        # input scale s = max(amax, floor) / 448 and its exact reciprocal
        s_p = singles.tile([P, 1], F32)
        nc.vector.tensor_scalar(
            s_p[:], gmaxP[:], AMAX_FLOOR, 1.0 / FP8_MAX, ALU.max, ALU.mult
        )
        inv_s = singles.tile([P, 1], F32)
        nc.vector.reciprocal(inv_s[:], s_p[:])

        # ---------------- Phase W: quantize + transpose weight --------------
        # wqT[kk, ko, o] = w_q[o, ko*128+kk]   (fp16, resident, 8 MB)
        wqT = singles.tile([P, KO, o_dim], F16)
        wsc = singles.tile([P, OJ], F32)  # w scale; [p, j] <-> o = j*128+p
        for j in range(OJ):
            wt = win.tile([P, k_dim], F32, tag="wt")
            nc.sync.dma_start(wt[:], w[ts(j, P), :])
            wam = small.tile([P, 1], F32, tag="wam")
            nc.vector.tensor_reduce(
                wam[:], wt[:], axis=AX.X, op=ALU.max, apply_absolute_value=True
            )
            nc.vector.tensor_scalar(
                wsc[:, j : j + 1], wam[:], AMAX_FLOOR, 1.0 / FP8_MAX,
                ALU.max, ALU.mult,
            )
            winv = small.tile([P, 1], F32, tag="winv")
            nc.vector.reciprocal(winv[:], wsc[:, j : j + 1])
            wq = q16.tile([P, k_dim], F16, tag="q16buf", name="wq")
            _quantize(nc, pools, wt, wq, winv[:])
            nc.sync.dma_start_transpose(wqT[:, :, ts(j, P)], wq[:])

        # ---------------- combined per-o scale row + bias, broadcast --------
        # cs[p, j] = s * ws[p, j]; transpose via PE to get an o-contiguous row
        csc = singles.tile([P, OJ], F32)
        nc.vector.tensor_scalar(csc[:], wsc[:], s_p[:], None, ALU.mult)
        cst_ps = psum1.tile([OJ, P], F32, name="cst_ps", tag="psmisc")
        nc.tensor.transpose(cst_ps[:], csc[:], ident[:])
        cst = singles.tile([OJ, P], F32)
        nc.vector.tensor_copy(cst[:], cst_ps[:])
        # o-contiguous scale row -> DRAM bounce -> broadcast to all partitions
        cs_dram = nc.dram_tensor("cs_dram", [o_dim], F32)
        nc.sync.dma_start(cs_dram.ap().rearrange("(j c) -> j c", j=OJ), cst[:])
        cs_b = singles.tile([P, o_dim], F32)
        nc.sync.dma_start(
            cs_b[:],
            cs_dram.ap().rearrange("(a o) -> a o", a=1).to_broadcast((P, o_dim)),
        )
        bias_b = singles.tile([P, o_dim], F32)
        nc.sync.dma_start(
            bias_b[:], b.rearrange("(a o) -> a o", a=1).to_broadcast((P, o_dim))
        )

        # ---------------- Phase M: quantize x, transpose, matmul ------------
        for tt in range(TT):
            xt = xin.tile([P, k_dim], F32, tag="xt")
            nc.sync.dma_start(xt[:], x[ts(tt, P), :])
            xq = q16.tile([P, k_dim], F16, tag="q16buf", name="xq")
            _quantize(nc, pools, xt, xq, inv_s[:])
            xqT = xqt.tile([P, KO, P], F16, tag="xqT")
            nc.sync.dma_start_transpose(xqT[:], xq[:])

            ps = [
                psum.tile([P, N_TILE], F32, tag="ps", name=f"ps_{tt}_{oo}")
                for oo in range(OO)
            ]
            for ko in range(KO):
                for oo in range(OO):
                    nc.tensor.matmul(
                        ps[oo][:],
                        lhsT=xqT[:, ko, :],
                        rhs=wqT[:, ko, ts(oo, N_TILE)],
                        start=(ko == 0),
                        stop=(ko == KO - 1),
                    )
            for oo in range(OO):
                ot = outp.tile([P, N_TILE], F32, tag="ot")
                nc.any.tensor_tensor(ot[:], ps[oo][:], cs_b[:, ts(oo, N_TILE)], ALU.mult)
                nc.any.tensor_tensor(ot[:], ot[:], bias_b[:, ts(oo, N_TILE)], ALU.add)
                nc.sync.dma_start(out[ts(tt, P), ts(oo, N_TILE)], ot[:])


_NC_CACHE = {}


def _get_nc():
    key = "full"
    if key not in _NC_CACHE:
        _NC_CACHE[key] = build_nc()
    return _NC_CACHE[key]


def kernel(x, weight, bias, _trace=False):
    B, S, K = x.shape
    O = weight.shape[0]
    n = 8
    t_local = (B * S) // n
    x2 = np.ascontiguousarray(x.reshape(B * S, K).astype(np.float32, copy=False))
    w = np.ascontiguousarray(weight.astype(np.float32, copy=False))
    bb = np.ascontiguousarray(bias.astype(np.float32, copy=False))
    in_maps = [
        {"x": x2[i * t_local : (i + 1) * t_local], "w": w, "b": bb} for i in range(n)
    ]
    nc = _get_nc()
    res = run_bass_kernel_spmd(nc, in_maps, core_ids=list(range(n)), trace=_trace)
    outs = [res.results[i]["out"] for i in range(n)]
    full = np.concatenate(outs, axis=0).reshape(B, S, O)
    if _trace:
        return full, res
    return full
